# revision 8
# baseline (speedup 1.0000x reference)
"""Trainium2 Bass kernel for fused multi-head attention (16 heads, d=64,
b=2, n=2048, h=1024) across 8 NeuronCores — v20.

Sharding: 2 heads x BOTH batches per core (core c owns heads {2c, 2c+1}),
post-attention Ulysses AllToAll into a row-sharded output projection
(identical to v1).

v3 changes vs v1 (285us baseline):
- AV/denominator scheme identical to v1 (65-wide V-aug with a ones
  column; GpSimd cross-lane reduction measured 128us/tile on HW, so
  denominators stay on the PE).
- Prologue: host pre-arranges x/weights so every staging DMA is a
  contiguous 2D copy; the first-needed pieces (wqk-K, cos/sin low,
  x(b0,sc0)) ride two HWDGE queues and gate the first matmul at ~5us
  instead of 39us. Bulk x rides SWDGE behind them.
- Position order (0,0),(0,1),(1,0),(1,1),(0,2),(1,2),(0,3),(1,3): each
  AllToAll piece qc is emitted as soon as norm(0,qc)+norm(1,qc) are
  done (positions 4,5,7 and tail), so the serial ~30us collectives
  overlap the attention stream and only piece 3 remains in the tail.
"""

import sys

if "/opt/trn_rl_repo" not in sys.path:
    sys.path.insert(0, "/opt/trn_rl_repo")

import numpy as np
import ml_dtypes

import concourse.bass as bass
import concourse.mybir as mybir
import concourse.tile as tile
from concourse import bacc
from concourse import bass_isa
from concourse.bass import ts
from concourse.bass_utils import run_bass_kernel_spmd

BF16 = mybir.dt.bfloat16
F32 = mybir.dt.float32
ADD = mybir.AluOpType.add
MULT = mybir.AluOpType.mult
BYPASS = mybir.AluOpType.bypass
EXP = mybir.ActivationFunctionType.Exp
AXC = mybir.AxisListType.C

HEADS, D, H, N, B = 16, 64, 1024, 2048, 2
NC_ = 8
LH = 2            # local heads per core (one pair, both batches)
KC = 16           # k chunks of 128 over n=2048
QC = 4            # q chunks of 512 over n=2048 (= AllToAll pieces)
SC = 4            # x staging chunks of 512 positions
LQK = LH * D      # 128 local q (or k) columns
GK, GQ = 0, 1     # wqk group index: [K | Q]


def build_nc():
    nc = bacc.Bacc("TRN2", target_bir_lowering=False, debug=False, num_devices=NC_)

    # x host-staged [p, b, sc, hk, 512] so each (b,sc) chunk is one
    # contiguous 2D copy (8KB per partition)
    xT = nc.declare_dram_parameter("xT", [128, B * SC * 8 * 512], BF16,
                                   isOutput=False)
    # [p, g(K|Q), hk, 128]
    wqk = nc.declare_dram_parameter("wqk", [128, 2 * 8 * 128], BF16,
                                    isOutput=False)
    # [p, hk, 130] (65-wide per head: V columns + ones)
    wv = nc.declare_dram_parameter("wv", [128, 8 * 130], BF16, isOutput=False)
    # [p, hc, 1024]
    wout = nc.declare_dram_parameter("wout", [128, 8 * H], BF16, isOutput=False)
    cos2 = nc.declare_dram_parameter("cos2", [64, N], BF16, isOutput=False)
    # sinm[p] = sin value read at SOURCE partition p during the shuffle:
    # p%64 < 32 -> +sin[p%64+32], else -sin[p%64-32]
    sinm = nc.declare_dram_parameter("sinm", [64, N], BF16, isOutput=False)
    out = nc.declare_dram_parameter("out", [QC, 128, H], BF16, isOutput=True)

    with tile.TileContext(nc) as tc:
        with (
            tc.tile_pool(name="dram", bufs=1, space="DRAM") as dram,
            tc.tile_pool(name="sb", bufs=1) as sb,
            tc.tile_pool(name="sbw", bufs=1) as sbw,
            tc.tile_pool(name="psum", bufs=2, space="PSUM") as ps,
        ):
            a2a_in = [dram.tile([8, 128, 128], BF16, name=f"ain{i}")
                      for i in range(QC)]
            a2a_out = [dram.tile([8, 128, 128], BF16, name=f"aout{i}")
                       for i in range(QC)]

            # warmup collective first: absorbs the one-time CC barrier
            # under the staging/projection prologue
            warm_in = dram.tile([8, 128], BF16, name="warm_in")
            warm_out = dram.tile([8, 128], BF16, name="warm_out")
            warm_sb = sbw.tile([1, 128], BF16)
            nc.vector.memset(warm_sb[:, :], 0.0)
            nc.sync.dma_start(warm_in[0:1, :], warm_sb[:, :])
            nc.gpsimd.collective_compute(
                "AllToAll", BYPASS, replica_groups=[list(range(8))],
                ins=[warm_in.opt()], outs=[warm_out.opt()])

            # ---- staging ----
            xt_sb = sbw.tile([128, B * SC * 8 * 512], BF16)  # [p,b,sc,hk,j]
            wqk_sb = sbw.tile([128, 2 * 8 * 128], BF16)      # [p,g,hk,c]
            wv_sb = sbw.tile([128, 8 * 130], BF16)           # [p,hk,c]
            ones_sb = sbw.tile([1, 64], BF16)
            wout_sb = sbw.tile([128, 8 * H], BF16)
            cos2_sb = sbw.tile([128, N], BF16)
            sinm_sb = sbw.tile([128, N], BF16)

            def xt_chunk(b, sc):
                c0 = (b * SC + sc) * 4096
                return slice(c0, c0 + 4096)

            # critical bytes lead each FIFO queue; batch-1 x queues
            # BEHIND them so bulk traffic cannot starve the gate.
            # sync: wqk-K, x(b0,sc0-1), then x(b1,sc0-1)
            # scalar: wqk-Q, cos/sin, wv, then x(b1,sc2-3)
            # gpsimd SWDGE: x(b0,sc2-3) only
            nc.sync.dma_start(wqk_sb[:, 0:1024], wqk[:, 0:1024])
            nc.gpsimd.dma_start(xt_sb[:, xt_chunk(0, 0)], xT[:, xt_chunk(0, 0)])
            nc.sync.dma_start(xt_sb[:, xt_chunk(0, 1)], xT[:, xt_chunk(0, 1)])
            nc.scalar.dma_start(wqk_sb[:, 1024:2048], wqk[:, 1024:2048])
            nc.scalar.dma_start(cos2_sb[0:64, 0:1024], cos2[:, 0:1024])
            nc.scalar.dma_start(sinm_sb[0:64, 0:1024], sinm[:, 0:1024])
            nc.scalar.dma_start(cos2_sb[0:64, 1024:2048], cos2[:, 1024:2048])
            nc.scalar.dma_start(sinm_sb[0:64, 1024:2048], sinm[:, 1024:2048])
            nc.gpsimd.dma_start(wv_sb[:, :], wv[:, :])
            nc.gpsimd.dma_start(xt_sb[:, 8192:16384], xT[:, 8192:16384])
            nc.sync.dma_start(xt_sb[:, xt_chunk(1, 0)], xT[:, xt_chunk(1, 0)])
            nc.sync.dma_start(xt_sb[:, xt_chunk(1, 1)], xT[:, xt_chunk(1, 1)])
            nc.scalar.dma_start(xt_sb[:, xt_chunk(1, 2)], xT[:, xt_chunk(1, 2)])
            nc.scalar.dma_start(xt_sb[:, xt_chunk(1, 3)], xT[:, xt_chunk(1, 3)])

            # duplicate cos/sin low halves to partitions 64-127 now; high
            # halves dup inside P0 so the DVE queue never blocks on their
            # later DMAs ahead of the prologue rotaries
            nc.vector.tensor_copy(cos2_sb[64:128, 0:1024],
                                  cos2_sb[0:64, 0:1024])
            nc.vector.tensor_copy(sinm_sb[64:128, 0:1024],
                                  sinm_sb[0:64, 0:1024])

            def dup_hi():
                nc.vector.tensor_copy(cos2_sb[64:128, 1024:2048],
                                      cos2_sb[0:64, 1024:2048])
                nc.vector.tensor_copy(sinm_sb[64:128, 1024:2048],
                                      sinm_sb[0:64, 1024:2048])

            kt_rot = sb.tile([128, B * N], BF16)   # [batch b at b*N][n]
            qt_rot = sb.tile([128, B * N], BF16)
            vt_all = sb.tile([128, B * KC * 130], BF16)  # [p, b, kc, 2h*65]
            # attn^T laid out [qc][b][row-block r][row-in-block]: shard
            # j = 4b+r of piece qc is the contiguous span (qc, b, r)
            attn_sb = sb.tile([128, B * N], BF16)
            attn4 = attn_sb.rearrange("p (q b r x) -> p q b r x", q=QC, b=B,
                                      x=128)

            nc.vector.memset(ones_sb[:, :], 1.0)
            # per-head ones columns of v-aug, set once
            nc.vector.memset(
                vt_all.rearrange("p (g e) -> p g e", e=65)[:, :, 64:65], 1.0)

            def proj_group(g, b, sc):
                p = ps.tile([128, 512], F32, tag="b", name="pp")
                for hk in range(8):
                    nc.tensor.matmul(
                        p[:, :],
                        lhsT=wqk_sb[:, (g * 8 + hk) * 128:][:, :128],
                        rhs=xt_sb[:, ((b * SC + sc) * 8 + hk) * 512:][:, :512],
                        start=(hk == 0),
                        stop=(hk == 7),
                    )
                return p

            def rotary_apply(psums, dst, pos0, W):
                """Rotary on a [128, W] span covering positions
                [pos0, pos0+W): stage psums to bf16 (DVE), then the
                partition-swapped sin multiply + cos multiply + add."""
                stage = sb.tile([128, 1024], BF16, tag="stg", bufs=2, name="stg")
                for i, p in enumerate(psums):
                    nc.vector.tensor_copy(stage[:, ts(i, 512)], p[:, :])
                tmp = sb.tile([128, 1024], BF16, tag="rta", bufs=2, name="rta")
                tmp2 = sb.tile([128, 1024], BF16, tag="rtb", bufs=2, name="rtb")
                sl = sinm_sb[:, pos0:pos0 + W]
                cl = cos2_sb[:, pos0:pos0 + W]
                for hh in (0, 64):
                    nc.vector.tensor_tensor(
                        tmp[hh:hh + 32, :W], stage[hh + 32:hh + 64, :W],
                        sl[hh + 32:hh + 64, :], MULT)
                    nc.vector.tensor_tensor(
                        tmp[hh + 32:hh + 64, :W], stage[hh:hh + 32, :W],
                        sl[hh:hh + 32, :], MULT)
                nc.vector.tensor_tensor(tmp2[:, :W], stage[:, :W], cl, MULT)
                nc.vector.tensor_tensor(dst, tmp2[:, :W], tmp[:, :W], ADD)

            def v_chunk(b, rc):
                p = ps.tile([128, 130], F32, tag="b", name="vp")
                sc, r = rc // 4, (rc % 4) * 128
                for hk in range(8):
                    nc.tensor.matmul(
                        p[:, :],
                        lhsT=xt_sb[:, ((b * SC + sc) * 8 + hk) * 512 + r:][:, :128],
                        rhs=wv_sb[:, ts(hk, 130)],
                        start=(hk == 0),
                        stop=(hk == 7),
                    )
                nc.vector.tensor_copy(
                    vt_all[:, (b * KC + rc) * 130:][:, :130].rearrange(
                        "p (h e) -> p h e", e=65)[:, :, 0:64],
                    p.rearrange("p (h e) -> p h e", e=65)[:, :, 0:64])

            def _av_mm(e, av0, av1, b, kc):
                base = (b * KC + kc) * 130
                nc.tensor.matmul(
                    av0[:, :], lhsT=vt_all[:, base:][:, :65],
                    rhs=e[:, 0:512], start=(kc == 0), stop=(kc == KC - 1))
                nc.tensor.matmul(
                    av1[:, :], lhsT=vt_all[:, base + 65:][:, :65],
                    rhs=e[:, 512:1024], start=(kc == 0), stop=(kc == KC - 1))

            def finish_copy(av0, av1):
                # move attn_out^T (+denominator row 64) out of PSUM right
                # away so the next position's AV accumulation can reuse the
                # banks; also stage both denominator rows at partition 0 for
                # the broadcast matmul. Normalization itself runs later.
                a0 = sb.tile([65, 512], BF16, tag="avs", bufs=6, name="a0")
                a1 = sb.tile([65, 512], BF16, tag="avs", bufs=6, name="a1")
                nc.vector.tensor_copy(a0[:, :], av0[:, :])
                nc.vector.tensor_copy(a1[:, :], av1[:, :])
                ad = sb.tile([1, 1024], BF16, tag="adn", bufs=2, name="ad")
                nc.vector.tensor_copy(ad[:, 0:512], a0[64:65, :])
                nc.vector.tensor_copy(ad[:, 512:1024], a1[64:65, :])
                return a0, a1, ad

            def finish_norm(qc, b, a0, a1, ad):
                b_ps = ps.tile([128, 512], F32, tag="b", name="b_ps")
                nc.tensor.matmul(b_ps[0:64, :], lhsT=ones_sb[:, :],
                                 rhs=ad[:, 0:512], start=True, stop=True,
                                 tile_position=(0, 0))
                nc.tensor.matmul(b_ps[64:128, :], lhsT=ones_sb[:, :],
                                 rhs=ad[:, 512:1024], start=True, stop=True,
                                 tile_position=(0, 64))
                bd_sb = sb.tile([128, 512], F32, tag="bsd", bufs=2, name="bd_sb")
                nc.vector.tensor_copy(bd_sb[:, :], b_ps[:, :])
                b_sb = sb.tile([128, 512], F32, tag="bsb", bufs=2, name="b_sb")
                nc.vector.reciprocal_approx_fast(out=b_sb[:, :], in_=bd_sb[:, :])
                # both TT inputs must share a base partition: bring head B's
                # inv-denominators down to partitions 0-63
                b_lo = sb.tile([64, 512], F32, tag="blo", bufs=2, name="b_lo")
                nc.vector.tensor_copy(b_lo[:, :], b_sb[64:128, :])
                dst = attn4[:, qc, b, :, :]  # [128, 4, 128]
                b3 = b_sb.rearrange("p (j x) -> p j x", x=128)
                bl3 = b_lo.rearrange("p (j x) -> p j x", x=128)
                nc.vector.tensor_tensor(dst[0:64], a0[0:64, :].rearrange(
                    "p (j x) -> p j x", x=128), b3[0:64], MULT)
                nc.vector.tensor_tensor(dst[64:128], a1[0:64, :].rearrange(
                    "p (j x) -> p j x", x=128), bl3[:, :, :], MULT)

            def a2a_send(qc, b):
                # shard j=4b+r carries my 2 heads for (batch b, row block r)
                # of piece qc; b=None sends both batches in one DMA
                src = attn4[:, qc, :, :, :]          # [128, 2, 4, 128]
                d = a2a_in[qc].rearrange("(b r) p x -> p b r x", b=B)
                if b is not None:
                    src = src[:, b:b + 1, :, :]
                    d = d[:, b:b + 1, :, :]
                nc.sync.dma_start(d, src)

            def a2a_go(qc):
                nc.gpsimd.collective_compute(
                    "AllToAll", BYPASS, replica_groups=[list(range(8))],
                    ins=[a2a_in[qc].opt()], outs=[a2a_out[qc].opt()])

            def emit_a2a(qc):
                a2a_send(qc, None)
                a2a_go(qc)

            # tail-only: shard i of a2a_out = peer i's heads {2i, 2i+1} for
            # my 128 rows -> directly the outproj stationary operand
            def outproj_recv(qc):
                # recv rides the gpsimd queue, emitted after every
                # collective trigger: a recv blocking that FIFO while its
                # collective completes can no longer starve the exp stream
                # on scalar or delay a later piece's send on sync
                att_r = sb.tile([128, 8 * 128], BF16, tag="attr", bufs=2,
                                name="att_r")
                nc.gpsimd.dma_start(
                    att_r.rearrange("p (i x) -> p i x", i=8),
                    a2a_out[qc].rearrange("i p x -> p i x"))
                return att_r

            def outproj_piece(qc, att_r):
                # both 512-column halves accumulate in lockstep: each
                # att_r stationary loads once and streams twice back-to-back
                g3 = att_r.rearrange("p (c x) -> p c x", x=128)
                o0 = ps.tile([128, 512], F32, tag="b", name="o0")
                o1 = ps.tile([128, 512], F32, tag="b", name="o1")
                for hc in range(8):
                    for nh, o in ((0, o0), (1, o1)):
                        nc.tensor.matmul(
                            o[:, :],
                            lhsT=g3[:, hc, :],
                            rhs=wout_sb[:, hc * H + nh * 512:][:, :512],
                            start=(hc == 0),
                            stop=(hc == 7),
                        )
                for nh, o in ((0, o0), (1, o1)):
                    ob = sb.tile([128, 512], BF16, tag="ob", bufs=3, name="ob")
                    nc.vector.tensor_copy(ob[:, :], o[:, :])
                    eng = nc.scalar if (qc == 3 and nh == 1) else nc.sync
                    eng.dma_start(out[qc, :, ts(nh, 512)], ob[:, :])

            def att_pos(qc, b, hooks, tail=False):
                qt_p = qt_rot[:, b * N + qc * 512:][:, :512]
                av0 = ps.tile([65, 512], F32, tag="av", name="av0")
                av1 = ps.tile([65, 512], F32, tag="av", name="av1")
                exps = []
                for kc in range(KC):
                    s_ps = ps.tile([128, 1024], F32, tag="s", name="s_ps")
                    nc.tensor.matmul(
                        s_ps[:, 0:512],
                        lhsT=kt_rot[0:64, b * N + kc * 128:][:, :128],
                        rhs=qt_p[0:64, :], start=True, stop=True,
                        tile_position=(0, 0))
                    nc.tensor.matmul(
                        s_ps[:, 512:1024],
                        lhsT=kt_rot[64:128, b * N + kc * 128:][:, :128],
                        rhs=qt_p[64:128, :], start=True, stop=True,
                        tile_position=(64, 0))
                    e = sb.tile([128, 1024], BF16, tag="exp", bufs=8, name="e")
                    nc.scalar.activation(e[:, :], s_ps[:, :], EXP, scale=0.125)
                    exps.append(e)
                    for f in hooks.get(kc, []):
                        f()
                    if kc > 1:
                        _av_mm(exps[kc - 2], av0, av1, b, kc - 2)
                _av_mm(exps[KC - 2], av0, av1, b, KC - 2)
                _av_mm(exps[KC - 1], av0, av1, b, KC - 1)
                if tail:
                    return av0, av1
                return finish_copy(av0, av1)

            # ---- prologue: only sc0 of K/Q (+V chunks 0-2) gate the
            # first scores; the rest arrives via hooks ----
            # all three projections emit before any rotary: k01's psum
            # WAR wait then pins to the stage cast right after it instead
            # of a threshold coarsened past two full rotary chains
            k00 = proj_group(GK, 0, 0)
            q00 = proj_group(GQ, 0, 0)
            k01 = proj_group(GK, 0, 1)
            rotary_apply([k00], kt_rot[:, 0:512], 0, 512)
            rotary_apply([q00], qt_rot[:, 0:512], 0, 512)
            rotary_apply([k01], kt_rot[:, 512:1024], 512, 512)

            # closure helpers for hook tables
            def mk(f, *a):
                return lambda: f(*a)

            grabs = {}

            def grab(key, g, b, sc):
                def f():
                    grabs[key] = proj_group(g, b, sc)
                return f

            def rotk(key, b, sc):
                # rotate one 512-wide K block of batch b
                def f():
                    rotary_apply([grabs.pop(key)],
                                 kt_rot[:, b * N + sc * 512:][:, :512],
                                 sc * 512, 512)
                return f

            def rotq(key, b, qc):
                def f():
                    rotary_apply([grabs.pop(key)],
                                 qt_rot[:, b * N + qc * 512:][:, :512],
                                 qc * 512, 512)
                return f

            pend = {}

            def norm(qc, b):
                def f():
                    a0, a1, ad = pend.pop((qc, b))
                    finish_norm(qc, b, a0, a1, ad)
                return f

            def wout_dma():
                nc.sync.dma_start(wout_sb[:, :], wout[:, :])

            def merge(*tables):
                h = {}
                for t in tables:
                    for k, fs in t:
                        h.setdefault(k, []).extend(fs)
                return h

            def vj(b, lo, hi):
                # JIT v chunks: chunk rc hooked at slot rc; its AV runs
                # after slot rc+1's hooks (one full slot of margin)
                return [(k, [mk(v_chunk, b, k)]) for k in range(lo, hi)]

            SEQ = [
                # (b, qc, hooks); norm(qc, b) finalizes an EARLIER position
                (0, 0, merge([
                    (0, [mk(v_chunk, 0, 0), dup_hi]),
                    (1, [mk(v_chunk, 0, 1), grab("k02", GK, 0, 2)]),
                    (2, [mk(v_chunk, 0, 2), rotk("k02", 0, 2)]),
                    (3, [grab("k03", GK, 0, 3)]),
                    (4, [rotk("k03", 0, 3)]),
                    (6, [grab("q01", GQ, 0, 1)]),
                    (7, [rotq("q01", 0, 1)]),
                ], vj(0, 3, 16))),
                (0, 1, {
                    0: [grab("k10", GK, 1, 0)],
                    1: [rotk("k10", 1, 0)],
                    2: [grab("k11", GK, 1, 1)],
                    3: [rotk("k11", 1, 1)],
                    4: [grab("k12", GK, 1, 2)],
                    5: [rotk("k12", 1, 2)],
                    6: [grab("k13", GK, 1, 3)],
                    7: [rotk("k13", 1, 3)],
                    8: [grab("q10", GQ, 1, 0)],
                    9: [rotq("q10", 1, 0)],
                    10: [mk(v_chunk, 1, 0), wout_dma],
                    11: [mk(v_chunk, 1, 1)],
                    12: [mk(v_chunk, 1, 2)],
                    13: [norm(0, 0)],
                }),
                (1, 0, merge([
                    (2, [norm(1, 0)]),
                    (6, [grab("q11", GQ, 1, 1)]),
                    (7, [rotq("q11", 1, 1)]),
                ], vj(1, 3, 16))),
                (1, 1, {
                    2: [norm(0, 1)],
                    3: [mk(emit_a2a, 0)],
                    6: [grab("q02", GQ, 0, 2)],
                    7: [rotq("q02", 0, 2)],
                }),
                (0, 2, {
                    2: [norm(1, 1)],
                    3: [mk(emit_a2a, 1)],
                    6: [grab("q12", GQ, 1, 2)],
                    7: [rotq("q12", 1, 2)],
                }),
                (1, 2, {
                    2: [norm(2, 0)],
                    6: [grab("q03", GQ, 0, 3)],
                    7: [rotq("q03", 0, 3)],
                }),
                (0, 3, {
                    2: [norm(2, 1)],
                    3: [mk(emit_a2a, 2)],
                    6: [grab("q13", GQ, 1, 3)],
                    7: [rotq("q13", 1, 3)],
                }),
                (1, 3, {
                    2: [norm(3, 0)],
                    3: [mk(a2a_send, 3, 0)],
                }),
            ]
            for i, (b, qc, hooks) in enumerate(SEQ):
                pend[(qc, b)] = att_pos(qc, b, hooks, tail=(i == len(SEQ) - 1))

            # tail: final normalization + piece-3 collective, then ALL
            # output projections (priority-pinned so the scheduler cannot
            # hoist collective-gated work into the attention stream)
            with tc.high_priority():
                av0, av1 = pend.pop((3, 1))
                ad = sb.tile([1, 1024], BF16, tag="adn", bufs=2, name="adt")
                nc.vector.tensor_copy(ad[:, 0:512], av0[64:65, :])
                nc.vector.tensor_copy(ad[:, 512:1024], av1[64:65, :])
                b_ps = ps.tile([128, 512], F32, tag="b", name="b_ps")
                nc.tensor.matmul(b_ps[0:64, :], lhsT=ones_sb[:, :],
                                 rhs=ad[:, 0:512], start=True, stop=True,
                                 tile_position=(0, 0))
                nc.tensor.matmul(b_ps[64:128, :], lhsT=ones_sb[:, :],
                                 rhs=ad[:, 512:1024], start=True, stop=True,
                                 tile_position=(0, 64))
                bd_sb = sb.tile([128, 512], F32, tag="bsd", bufs=2, name="bdt")
                nc.vector.tensor_copy(bd_sb[:, :], b_ps[:, :])
                b_sb = sb.tile([128, 512], F32, tag="bsb", bufs=2, name="bst")
                nc.vector.reciprocal_approx_fast(out=b_sb[:, :], in_=bd_sb[:, :])
                b_lo = sb.tile([64, 512], F32, tag="blo", bufs=2, name="blt")
                nc.vector.tensor_copy(b_lo[:, :], b_sb[64:128, :])
                dst = attn4[:, 3, 1, :, :]
                b3 = b_sb.rearrange("p (j x) -> p j x", x=128)
                bl3 = b_lo.rearrange("p (j x) -> p j x", x=128)
                nc.vector.tensor_tensor(dst[0:64], av0[0:64, :].rearrange(
                    "p (j x) -> p j x", x=128), b3[0:64], MULT)
                nc.vector.tensor_tensor(dst[64:128], av1[0:64, :].rearrange(
                    "p (j x) -> p j x", x=128), bl3[:, :, :], MULT)
                src31 = attn4[:, 3, 1:2, :, :]
                d31 = a2a_in[3].rearrange("(b r) p x -> p b r x", b=B)[:, 1:2]
                nc.sync.dma_start(d31[:, :, 0:2], src31[:, :, 0:2])
                nc.scalar.dma_start(d31[:, :, 2:4], src31[:, :, 2:4])
            a2a_go(3)
            with tc.high_priority(offset=-10_000_000):
                for qc in range(QC):
                    ar = outproj_recv(qc)
                    outproj_piece(qc, ar)

    nc.finalize()
    return nc


_NC = None


def _get_nc():
    global _NC
    if _NC is None:
        _NC = build_nc()
    return _NC


def _bf16(a):
    return np.ascontiguousarray(a.astype(ml_dtypes.bfloat16))


def make_in_maps(x, rotary_emb, w_qkv, w_out):
    x = np.asarray(x, np.float32)
    rotary_emb = np.asarray(rotary_emb, np.float32)
    w_qkv = np.asarray(w_qkv, np.float32)
    w_out = np.asarray(w_out, np.float32)
    cosT = np.cos(rotary_emb).T.astype(np.float32)  # [64, N]
    sinT = np.sin(rotary_emb).T.astype(np.float32)
    sswp = np.concatenate([sinT[32:], -sinT[:32]], axis=0)
    cos2_a = _bf16(cosT)
    sinm_a = _bf16(sswp)
    # wout [p, hc, 1024]
    wout_a = _bf16(w_out.reshape(8, 128, H).transpose(1, 0, 2).reshape(128, -1))
    # x [p, b, sc, hk, 512]
    xT_a = _bf16(x.reshape(B, SC, 512, 8, 128).transpose(4, 0, 1, 3, 2)
                 .reshape(128, -1))
    in_maps = []
    for c in range(NC_):
        h0 = LH * c  # heads {2c, 2c+1}
        wq_loc = w_qkv[:, 64 * h0: 64 * h0 + LQK]
        wk_loc = w_qkv[:, H + 64 * h0: H + 64 * h0 + LQK]
        wv_loc = w_qkv[:, 2 * H + 64 * h0: 2 * H + 64 * h0 + LQK]
        wv_aug = np.zeros((H, 130), np.float32)
        for j in range(LH):
            wv_aug[:, 65 * j: 65 * j + 64] = wv_loc[:, 64 * j: 64 * j + 64]
        # [p, g(K|Q), hk, 128]
        wqk_g = np.stack([wk_loc.reshape(8, 128, 128),
                          wq_loc.reshape(8, 128, 128)], axis=0)
        wqk_a = _bf16(wqk_g.transpose(2, 0, 1, 3).reshape(128, -1))
        wv_a = _bf16(wv_aug.reshape(8, 128, 130).transpose(1, 0, 2)
                     .reshape(128, -1))
        in_maps.append({
            "xT": xT_a,
            "wqk": wqk_a,
            "wv": wv_a,
            "wout": wout_a,
            "cos2": cos2_a,
            "sinm": sinm_a,
        })
    return in_maps


def run(x, rotary_emb, w_qkv, w_out, trace=False, tmpdir=None):
    nc = _get_nc()
    in_maps = make_in_maps(x, rotary_emb, w_qkv, w_out)
    res = run_bass_kernel_spmd(nc, in_maps, list(range(NC_)), trace=trace,
                               tmpdir=tmpdir)
    full = np.empty((B, N, H), np.float32)
    for c in range(NC_):
        b, r = c // 4, c % 4
        piece = np.asarray(res.results[c]["out"]).astype(np.float32)
        for qc in range(QC):
            full[b, 512 * qc + 128 * r: 512 * qc + 128 * r + 128] = piece[qc]
    return full, res


def kernel(x, rotary_emb, w_qkv, w_out):
    full, _ = run(x, rotary_emb, w_qkv, w_out)
    return full


# revision 9
# speedup vs baseline: 1.0472x; 1.0472x over previous
"""Trainium2 Bass kernel for fused multi-head attention (16 heads, d=64,
b=2, n=2048, h=1024) across 8 NeuronCores — v22.

Sharding: 2 heads x BOTH batches per core (core c owns heads {2c, 2c+1}),
post-attention Ulysses AllToAll into a row-sharded output projection
(identical to v1).

v3 changes vs v1 (285us baseline):
- AV/denominator scheme identical to v1 (65-wide V-aug with a ones
  column; GpSimd cross-lane reduction measured 128us/tile on HW, so
  denominators stay on the PE).
- Prologue: host pre-arranges x/weights so every staging DMA is a
  contiguous 2D copy; the first-needed pieces (wqk-K, cos/sin low,
  x(b0,sc0)) ride two HWDGE queues and gate the first matmul at ~5us
  instead of 39us. Bulk x rides SWDGE behind them.
- Position order (0,0),(0,1),(1,0),(1,1),(0,2),(1,2),(0,3),(1,3): each
  AllToAll piece qc is emitted as soon as norm(0,qc)+norm(1,qc) are
  done (positions 4,5,7 and tail), so the serial ~30us collectives
  overlap the attention stream and only piece 3 remains in the tail.
"""

import sys

if "/opt/trn_rl_repo" not in sys.path:
    sys.path.insert(0, "/opt/trn_rl_repo")

import numpy as np
import ml_dtypes

import concourse.bass as bass
import concourse.mybir as mybir
import concourse.tile as tile
from concourse import bacc
from concourse import bass_isa
from concourse.bass import ts
from concourse.bass_utils import run_bass_kernel_spmd

BF16 = mybir.dt.bfloat16
F32 = mybir.dt.float32
ADD = mybir.AluOpType.add
MULT = mybir.AluOpType.mult
BYPASS = mybir.AluOpType.bypass
EXP = mybir.ActivationFunctionType.Exp
AXC = mybir.AxisListType.C

HEADS, D, H, N, B = 16, 64, 1024, 2048, 2
NC_ = 8
LH = 2            # local heads per core (one pair, both batches)
KC = 16           # k chunks of 128 over n=2048
QC = 4            # q chunks of 512 over n=2048 (= AllToAll pieces)
SC = 4            # x staging chunks of 512 positions
LQK = LH * D      # 128 local q (or k) columns
GK, GQ = 0, 1     # wqk group index: [K | Q]


def build_nc():
    nc = bacc.Bacc("TRN2", target_bir_lowering=False, debug=False, num_devices=NC_)

    # x host-staged [p, b, sc, hk, 512] so each (b,sc) chunk is one
    # contiguous 2D copy (8KB per partition)
    xT = nc.declare_dram_parameter("xT", [128, B * SC * 8 * 512], BF16,
                                   isOutput=False)
    # [p, g(K|Q), hk, 128]
    wqk = nc.declare_dram_parameter("wqk", [128, 2 * 8 * 128], BF16,
                                    isOutput=False)
    # [p, hk, 130] (65-wide per head: V columns + ones)
    wv = nc.declare_dram_parameter("wv", [128, 8 * 130], BF16, isOutput=False)
    # [p, hc, 1024]
    wout = nc.declare_dram_parameter("wout", [128, 8 * H], BF16, isOutput=False)
    cos2 = nc.declare_dram_parameter("cos2", [64, N], BF16, isOutput=False)
    # sinm[p] = sin value read at SOURCE partition p during the shuffle:
    # p%64 < 32 -> +sin[p%64+32], else -sin[p%64-32]
    sinm = nc.declare_dram_parameter("sinm", [64, N], BF16, isOutput=False)
    out = nc.declare_dram_parameter("out", [QC, 128, H], BF16, isOutput=True)

    with tile.TileContext(nc) as tc:
        with (
            tc.tile_pool(name="dram", bufs=1, space="DRAM") as dram,
            tc.tile_pool(name="sb", bufs=1) as sb,
            tc.tile_pool(name="sbw", bufs=1) as sbw,
            tc.tile_pool(name="psum", bufs=2, space="PSUM") as ps,
        ):
            a2a_in = [dram.tile([8, 128, 128], BF16, name=f"ain{i}")
                      for i in range(QC)]
            a2a_out = [dram.tile([8, 128, 128], BF16, name=f"aout{i}")
                       for i in range(QC)]

            # warmup collective first: absorbs the one-time CC barrier
            # under the staging/projection prologue
            warm_in = dram.tile([8, 128], BF16, name="warm_in")
            warm_out = dram.tile([8, 128], BF16, name="warm_out")
            warm_sb = sbw.tile([1, 128], BF16)
            nc.vector.memset(warm_sb[:, :], 0.0)
            nc.sync.dma_start(warm_in[0:1, :], warm_sb[:, :])
            nc.gpsimd.collective_compute(
                "AllToAll", BYPASS, replica_groups=[list(range(8))],
                ins=[warm_in.opt()], outs=[warm_out.opt()])

            # ---- staging ----
            xt_sb = sbw.tile([128, B * SC * 8 * 512], BF16)  # [p,b,sc,hk,j]
            wqk_sb = sbw.tile([128, 2 * 8 * 128], BF16)      # [p,g,hk,c]
            wv_sb = sbw.tile([128, 8 * 130], BF16)           # [p,hk,c]
            ones_sb = sbw.tile([1, 64], BF16)
            wout_sb = sbw.tile([128, 8 * H], BF16)
            cos2_sb = sbw.tile([128, N], BF16)
            sinm_sb = sbw.tile([128, N], BF16)

            def xt_chunk(b, sc):
                c0 = (b * SC + sc) * 4096
                return slice(c0, c0 + 4096)

            # critical bytes lead each FIFO queue; batch-1 x queues
            # BEHIND them so bulk traffic cannot starve the gate.
            # sync: wqk-K, x(b0,sc0-1), then x(b1,sc0-1)
            # scalar: wqk-Q, cos/sin, wv, then x(b1,sc2-3)
            # gpsimd SWDGE: x(b0,sc2-3) only
            nc.sync.dma_start(wqk_sb[:, 0:1024], wqk[:, 0:1024])
            nc.gpsimd.dma_start(xt_sb[:, xt_chunk(0, 0)], xT[:, xt_chunk(0, 0)])
            nc.sync.dma_start(xt_sb[:, xt_chunk(0, 1)], xT[:, xt_chunk(0, 1)])
            nc.scalar.dma_start(wqk_sb[:, 1024:2048], wqk[:, 1024:2048])
            nc.scalar.dma_start(cos2_sb[0:64, 0:1024], cos2[:, 0:1024])
            nc.scalar.dma_start(sinm_sb[0:64, 0:1024], sinm[:, 0:1024])
            nc.scalar.dma_start(cos2_sb[0:64, 1024:2048], cos2[:, 1024:2048])
            nc.scalar.dma_start(sinm_sb[0:64, 1024:2048], sinm[:, 1024:2048])
            nc.gpsimd.dma_start(wv_sb[:, :], wv[:, :])
            nc.gpsimd.dma_start(xt_sb[:, 8192:16384], xT[:, 8192:16384])
            nc.sync.dma_start(xt_sb[:, xt_chunk(1, 0)], xT[:, xt_chunk(1, 0)])
            nc.sync.dma_start(xt_sb[:, xt_chunk(1, 1)], xT[:, xt_chunk(1, 1)])
            nc.scalar.dma_start(xt_sb[:, xt_chunk(1, 2)], xT[:, xt_chunk(1, 2)])
            nc.scalar.dma_start(xt_sb[:, xt_chunk(1, 3)], xT[:, xt_chunk(1, 3)])

            # duplicate cos/sin low halves to partitions 64-127 now; high
            # halves dup inside P0 so the DVE queue never blocks on their
            # later DMAs ahead of the prologue rotaries
            nc.vector.tensor_copy(cos2_sb[64:128, 0:1024],
                                  cos2_sb[0:64, 0:1024])
            nc.vector.tensor_copy(sinm_sb[64:128, 0:1024],
                                  sinm_sb[0:64, 0:1024])

            def dup_hi():
                nc.vector.tensor_copy(cos2_sb[64:128, 1024:2048],
                                      cos2_sb[0:64, 1024:2048])
                nc.vector.tensor_copy(sinm_sb[64:128, 1024:2048],
                                      sinm_sb[0:64, 1024:2048])

            kt_rot = sb.tile([128, B * N], BF16)   # [batch b at b*N][n]
            qt_rot = sb.tile([128, B * N], BF16)
            vt_all = sb.tile([128, B * KC * 130], BF16)  # [p, b, kc, 2h*65]
            # attn^T laid out [qc][b][row-block r][row-in-block]: shard
            # j = 4b+r of piece qc is the contiguous span (qc, b, r)
            attn_sb = sb.tile([128, B * N], BF16)
            attn4 = attn_sb.rearrange("p (q b r x) -> p q b r x", q=QC, b=B,
                                      x=128)

            nc.vector.memset(ones_sb[:, :], 1.0)
            # per-head ones columns of v-aug, set once
            nc.vector.memset(
                vt_all.rearrange("p (g e) -> p g e", e=65)[:, :, 64:65], 1.0)

            def proj_group(g, b, sc):
                p = ps.tile([128, 512], F32, tag="b", name="pp")
                for hk in range(8):
                    nc.tensor.matmul(
                        p[:, :],
                        lhsT=wqk_sb[:, (g * 8 + hk) * 128:][:, :128],
                        rhs=xt_sb[:, ((b * SC + sc) * 8 + hk) * 512:][:, :512],
                        start=(hk == 0),
                        stop=(hk == 7),
                    )
                return p

            def rotary_apply(psums, dst, pos0, W):
                """Rotary on a [128, W] span covering positions
                [pos0, pos0+W): stage psums to bf16 (DVE), then the
                partition-swapped sin multiply + cos multiply + add."""
                stage = sb.tile([128, 1024], BF16, tag="stg", bufs=2, name="stg")
                for i, p in enumerate(psums):
                    nc.vector.tensor_copy(stage[:, ts(i, 512)], p[:, :])
                tmp = sb.tile([128, 1024], BF16, tag="rta", bufs=2, name="rta")
                tmp2 = sb.tile([128, 1024], BF16, tag="rtb", bufs=2, name="rtb")
                sl = sinm_sb[:, pos0:pos0 + W]
                cl = cos2_sb[:, pos0:pos0 + W]
                for hh in (0, 64):
                    nc.vector.tensor_tensor(
                        tmp[hh:hh + 32, :W], stage[hh + 32:hh + 64, :W],
                        sl[hh + 32:hh + 64, :], MULT)
                    nc.vector.tensor_tensor(
                        tmp[hh + 32:hh + 64, :W], stage[hh:hh + 32, :W],
                        sl[hh:hh + 32, :], MULT)
                nc.vector.tensor_tensor(tmp2[:, :W], stage[:, :W], cl, MULT)
                nc.vector.tensor_tensor(dst, tmp2[:, :W], tmp[:, :W], ADD)

            def v_chunk(b, rc):
                p = ps.tile([128, 130], F32, tag="b", name="vp")
                sc, r = rc // 4, (rc % 4) * 128
                for hk in range(8):
                    nc.tensor.matmul(
                        p[:, :],
                        lhsT=xt_sb[:, ((b * SC + sc) * 8 + hk) * 512 + r:][:, :128],
                        rhs=wv_sb[:, ts(hk, 130)],
                        start=(hk == 0),
                        stop=(hk == 7),
                    )
                nc.vector.tensor_copy(
                    vt_all[:, (b * KC + rc) * 130:][:, :130].rearrange(
                        "p (h e) -> p h e", e=65)[:, :, 0:64],
                    p.rearrange("p (h e) -> p h e", e=65)[:, :, 0:64])

            def _av_mm(e, av0, av1, b, kc):
                base = (b * KC + kc) * 130
                nc.tensor.matmul(
                    av0[:, :], lhsT=vt_all[:, base:][:, :65],
                    rhs=e[:, 0:512], start=(kc == 0), stop=(kc == KC - 1))
                nc.tensor.matmul(
                    av1[:, :], lhsT=vt_all[:, base + 65:][:, :65],
                    rhs=e[:, 512:1024], start=(kc == 0), stop=(kc == KC - 1))

            def finish_copy(av0, av1):
                # move attn_out^T (+denominator row 64) out of PSUM right
                # away so the next position's AV accumulation can reuse the
                # banks; also stage both denominator rows at partition 0 for
                # the broadcast matmul. Normalization itself runs later.
                a0 = sb.tile([65, 512], BF16, tag="avs", bufs=6, name="a0")
                a1 = sb.tile([65, 512], BF16, tag="avs", bufs=6, name="a1")
                nc.vector.tensor_copy(a0[:, :], av0[:, :])
                nc.vector.tensor_copy(a1[:, :], av1[:, :])
                ad = sb.tile([1, 1024], BF16, tag="adn", bufs=2, name="ad")
                nc.vector.tensor_copy(ad[:, 0:512], a0[64:65, :])
                nc.vector.tensor_copy(ad[:, 512:1024], a1[64:65, :])
                return a0, a1, ad

            def finish_norm(qc, b, a0, a1, ad):
                b_ps = ps.tile([128, 512], F32, tag="b", name="b_ps")
                nc.tensor.matmul(b_ps[0:64, :], lhsT=ones_sb[:, :],
                                 rhs=ad[:, 0:512], start=True, stop=True,
                                 tile_position=(0, 0))
                nc.tensor.matmul(b_ps[64:128, :], lhsT=ones_sb[:, :],
                                 rhs=ad[:, 512:1024], start=True, stop=True,
                                 tile_position=(0, 64))
                bd_sb = sb.tile([128, 512], F32, tag="bsd", bufs=2, name="bd_sb")
                nc.vector.tensor_copy(bd_sb[:, :], b_ps[:, :])
                b_sb = sb.tile([128, 512], F32, tag="bsb", bufs=2, name="b_sb")
                nc.vector.reciprocal_approx_fast(out=b_sb[:, :], in_=bd_sb[:, :])
                # both TT inputs must share a base partition: bring head B's
                # inv-denominators down to partitions 0-63
                b_lo = sb.tile([64, 512], F32, tag="blo", bufs=2, name="b_lo")
                nc.vector.tensor_copy(b_lo[:, :], b_sb[64:128, :])
                dst = attn4[:, qc, b, :, :]  # [128, 4, 128]
                b3 = b_sb.rearrange("p (j x) -> p j x", x=128)
                bl3 = b_lo.rearrange("p (j x) -> p j x", x=128)
                nc.vector.tensor_tensor(dst[0:64], a0[0:64, :].rearrange(
                    "p (j x) -> p j x", x=128), b3[0:64], MULT)
                nc.vector.tensor_tensor(dst[64:128], a1[0:64, :].rearrange(
                    "p (j x) -> p j x", x=128), bl3[:, :, :], MULT)

            def a2a_send(qc, b):
                # shard j=4b+r carries my 2 heads for (batch b, row block r)
                # of piece qc; b=None sends both batches in one DMA
                src = attn4[:, qc, :, :, :]          # [128, 2, 4, 128]
                d = a2a_in[qc].rearrange("(b r) p x -> p b r x", b=B)
                if b is not None:
                    src = src[:, b:b + 1, :, :]
                    d = d[:, b:b + 1, :, :]
                nc.sync.dma_start(d, src)

            def a2a_go(qc):
                nc.gpsimd.collective_compute(
                    "AllToAll", BYPASS, replica_groups=[list(range(8))],
                    ins=[a2a_in[qc].opt()], outs=[a2a_out[qc].opt()])

            def emit_a2a(qc):
                a2a_send(qc, None)
                a2a_go(qc)

            # tail-only: shard i of a2a_out = peer i's heads {2i, 2i+1} for
            # my 128 rows -> directly the outproj stationary operand
            def outproj_recv(qc):
                # recv rides the gpsimd queue, emitted after every
                # collective trigger: a recv blocking that FIFO while its
                # collective completes can no longer starve the exp stream
                # on scalar or delay a later piece's send on sync
                att_r = sb.tile([128, 8 * 128], BF16, tag="attr", bufs=2,
                                name="att_r")
                av = att_r.rearrange("p (i x) -> p i x", i=8)
                sv = a2a_out[qc].rearrange("i p x -> p i x")
                if qc == 3:
                    # tail-critical recv: skip the SWDGE setup overhead and
                    # pull both halves on the (idle) HWDGE queues; all piece
                    # sends are already ahead of this point in their FIFOs
                    nc.sync.dma_start(av[:, 0:4], sv[:, 0:4])
                    nc.scalar.dma_start(av[:, 4:8], sv[:, 4:8])
                else:
                    nc.gpsimd.dma_start(av, sv)
                return att_r

            def outproj_piece(qc, att_r):
                # both 512-column halves accumulate in lockstep: each
                # att_r stationary loads once and streams twice back-to-back
                g3 = att_r.rearrange("p (c x) -> p c x", x=128)
                o0 = ps.tile([128, 512], F32, tag="b", name="o0")
                o1 = ps.tile([128, 512], F32, tag="b", name="o1")
                for hc in range(8):
                    for nh, o in ((0, o0), (1, o1)):
                        nc.tensor.matmul(
                            o[:, :],
                            lhsT=g3[:, hc, :],
                            rhs=wout_sb[:, hc * H + nh * 512:][:, :512],
                            start=(hc == 0),
                            stop=(hc == 7),
                        )
                for nh, o in ((0, o0), (1, o1)):
                    ob = sb.tile([128, 512], BF16, tag="ob", bufs=3, name="ob")
                    nc.vector.tensor_copy(ob[:, :], o[:, :])
                    eng = nc.scalar if (qc == 3 and nh == 1) else nc.sync
                    eng.dma_start(out[qc, :, ts(nh, 512)], ob[:, :])

            def att_pos(qc, b, hooks, tail=False):
                qt_p = qt_rot[:, b * N + qc * 512:][:, :512]
                av0 = ps.tile([65, 512], F32, tag="av", name="av0")
                av1 = ps.tile([65, 512], F32, tag="av", name="av1")
                exps = []
                for kc in range(KC):
                    s_ps = ps.tile([128, 1024], F32, tag="s", name="s_ps")
                    nc.tensor.matmul(
                        s_ps[:, 0:512],
                        lhsT=kt_rot[0:64, b * N + kc * 128:][:, :128],
                        rhs=qt_p[0:64, :], start=True, stop=True,
                        tile_position=(0, 0))
                    nc.tensor.matmul(
                        s_ps[:, 512:1024],
                        lhsT=kt_rot[64:128, b * N + kc * 128:][:, :128],
                        rhs=qt_p[64:128, :], start=True, stop=True,
                        tile_position=(64, 0))
                    e = sb.tile([128, 1024], BF16, tag="exp", bufs=8, name="e")
                    nc.scalar.activation(e[:, :], s_ps[:, :], EXP, scale=0.125)
                    exps.append(e)
                    for f in hooks.get(kc, []):
                        f()
                    if kc > 1:
                        _av_mm(exps[kc - 2], av0, av1, b, kc - 2)
                _av_mm(exps[KC - 2], av0, av1, b, KC - 2)
                _av_mm(exps[KC - 1], av0, av1, b, KC - 1)
                if tail:
                    return av0, av1
                return finish_copy(av0, av1)

            # ---- prologue: only sc0 of K/Q (+V chunks 0-2) gate the
            # first scores; the rest arrives via hooks ----
            # all three projections emit before any rotary: k01's psum
            # WAR wait then pins to the stage cast right after it instead
            # of a threshold coarsened past two full rotary chains
            k00 = proj_group(GK, 0, 0)
            q00 = proj_group(GQ, 0, 0)
            k01 = proj_group(GK, 0, 1)
            rotary_apply([k00], kt_rot[:, 0:512], 0, 512)
            rotary_apply([q00], qt_rot[:, 0:512], 0, 512)
            rotary_apply([k01], kt_rot[:, 512:1024], 512, 512)

            # closure helpers for hook tables
            def mk(f, *a):
                return lambda: f(*a)

            grabs = {}

            def grab(key, g, b, sc):
                def f():
                    grabs[key] = proj_group(g, b, sc)
                return f

            def rotk(key, b, sc):
                # rotate one 512-wide K block of batch b
                def f():
                    rotary_apply([grabs.pop(key)],
                                 kt_rot[:, b * N + sc * 512:][:, :512],
                                 sc * 512, 512)
                return f

            def rotq(key, b, qc):
                def f():
                    rotary_apply([grabs.pop(key)],
                                 qt_rot[:, b * N + qc * 512:][:, :512],
                                 qc * 512, 512)
                return f

            pend = {}

            def norm(qc, b):
                def f():
                    a0, a1, ad = pend.pop((qc, b))
                    finish_norm(qc, b, a0, a1, ad)
                return f

            def wout_dma():
                nc.sync.dma_start(wout_sb[:, :], wout[:, :])

            def merge(*tables):
                h = {}
                for t in tables:
                    for k, fs in t:
                        h.setdefault(k, []).extend(fs)
                return h

            def vj(b, lo, hi):
                # JIT v chunks: chunk rc hooked at slot rc; its AV runs
                # after slot rc+1's hooks (one full slot of margin)
                return [(k, [mk(v_chunk, b, k)]) for k in range(lo, hi)]

            SEQ = [
                # (b, qc, hooks); norm(qc, b) finalizes an EARLIER position
                (0, 0, merge([
                    (0, [mk(v_chunk, 0, 0), dup_hi]),
                    (1, [mk(v_chunk, 0, 1), grab("k02", GK, 0, 2)]),
                    (2, [mk(v_chunk, 0, 2), rotk("k02", 0, 2)]),
                    (3, [grab("k03", GK, 0, 3)]),
                    (4, [rotk("k03", 0, 3)]),
                    (6, [grab("q01", GQ, 0, 1)]),
                    (7, [rotq("q01", 0, 1)]),
                ], vj(0, 3, 16))),
                (0, 1, {
                    0: [grab("k10", GK, 1, 0)],
                    1: [rotk("k10", 1, 0)],
                    2: [grab("k11", GK, 1, 1)],
                    3: [rotk("k11", 1, 1)],
                    4: [grab("k12", GK, 1, 2)],
                    5: [rotk("k12", 1, 2)],
                    6: [grab("k13", GK, 1, 3)],
                    7: [rotk("k13", 1, 3)],
                    8: [grab("q10", GQ, 1, 0)],
                    9: [rotq("q10", 1, 0)],
                    10: [mk(v_chunk, 1, 0), wout_dma],
                    11: [mk(v_chunk, 1, 1)],
                    12: [mk(v_chunk, 1, 2)],
                    13: [norm(0, 0)],
                }),
                (1, 0, merge([
                    (2, [norm(1, 0)]),
                    (6, [grab("q11", GQ, 1, 1)]),
                    (7, [rotq("q11", 1, 1)]),
                ], vj(1, 3, 16))),
                (1, 1, {
                    2: [norm(0, 1)],
                    3: [mk(emit_a2a, 0)],
                    6: [grab("q02", GQ, 0, 2)],
                    7: [rotq("q02", 0, 2)],
                }),
                (0, 2, {
                    2: [norm(1, 1)],
                    3: [mk(emit_a2a, 1)],
                    6: [grab("q12", GQ, 1, 2)],
                    7: [rotq("q12", 1, 2)],
                }),
                (1, 2, {
                    2: [norm(2, 0)],
                    6: [grab("q03", GQ, 0, 3)],
                    7: [rotq("q03", 0, 3)],
                }),
                (0, 3, {
                    2: [norm(2, 1)],
                    3: [mk(emit_a2a, 2)],
                    6: [grab("q13", GQ, 1, 3)],
                    7: [rotq("q13", 1, 3)],
                }),
                (1, 3, {
                    2: [norm(3, 0)],
                    3: [mk(a2a_send, 3, 0)],
                }),
            ]
            for i, (b, qc, hooks) in enumerate(SEQ):
                pend[(qc, b)] = att_pos(qc, b, hooks, tail=(i == len(SEQ) - 1))

            # tail: final normalization + piece-3 collective, then ALL
            # output projections (priority-pinned so the scheduler cannot
            # hoist collective-gated work into the attention stream)
            with tc.high_priority():
                av0, av1 = pend.pop((3, 1))
                ad = sb.tile([1, 1024], BF16, tag="adn", bufs=2, name="adt")
                nc.vector.tensor_copy(ad[:, 0:512], av0[64:65, :])
                nc.vector.tensor_copy(ad[:, 512:1024], av1[64:65, :])
                b_ps = ps.tile([128, 512], F32, tag="b", name="b_ps")
                nc.tensor.matmul(b_ps[0:64, :], lhsT=ones_sb[:, :],
                                 rhs=ad[:, 0:512], start=True, stop=True,
                                 tile_position=(0, 0))
                nc.tensor.matmul(b_ps[64:128, :], lhsT=ones_sb[:, :],
                                 rhs=ad[:, 512:1024], start=True, stop=True,
                                 tile_position=(0, 64))
                bd_sb = sb.tile([128, 512], F32, tag="bsd", bufs=2, name="bdt")
                nc.vector.tensor_copy(bd_sb[:, :], b_ps[:, :])
                b_sb = sb.tile([128, 512], F32, tag="bsb", bufs=2, name="bst")
                nc.vector.reciprocal_approx_fast(out=b_sb[:, :], in_=bd_sb[:, :])
                b_lo = sb.tile([64, 512], F32, tag="blo", bufs=2, name="blt")
                nc.vector.tensor_copy(b_lo[:, :], b_sb[64:128, :])
                dst = attn4[:, 3, 1, :, :]
                b3 = b_sb.rearrange("p (j x) -> p j x", x=128)
                bl3 = b_lo.rearrange("p (j x) -> p j x", x=128)
                nc.vector.tensor_tensor(dst[0:64], av0[0:64, :].rearrange(
                    "p (j x) -> p j x", x=128), b3[0:64], MULT)
                nc.vector.tensor_tensor(dst[64:128], av1[0:64, :].rearrange(
                    "p (j x) -> p j x", x=128), bl3[:, :, :], MULT)
                src31 = attn4[:, 3, 1:2, :, :]
                d31 = a2a_in[3].rearrange("(b r) p x -> p b r x", b=B)[:, 1:2]
                nc.sync.dma_start(d31[:, :, 0:2], src31[:, :, 0:2])
                nc.scalar.dma_start(d31[:, :, 2:4], src31[:, :, 2:4])
            a2a_go(3)
            with tc.high_priority(offset=-10_000_000):
                for qc in range(QC):
                    ar = outproj_recv(qc)
                    outproj_piece(qc, ar)

    nc.finalize()
    return nc


_NC = None


def _get_nc():
    global _NC
    if _NC is None:
        _NC = build_nc()
    return _NC


def _bf16(a):
    return np.ascontiguousarray(a.astype(ml_dtypes.bfloat16))


def make_in_maps(x, rotary_emb, w_qkv, w_out):
    x = np.asarray(x, np.float32)
    rotary_emb = np.asarray(rotary_emb, np.float32)
    w_qkv = np.asarray(w_qkv, np.float32)
    w_out = np.asarray(w_out, np.float32)
    cosT = np.cos(rotary_emb).T.astype(np.float32)  # [64, N]
    sinT = np.sin(rotary_emb).T.astype(np.float32)
    sswp = np.concatenate([sinT[32:], -sinT[:32]], axis=0)
    cos2_a = _bf16(cosT)
    sinm_a = _bf16(sswp)
    # wout [p, hc, 1024]
    wout_a = _bf16(w_out.reshape(8, 128, H).transpose(1, 0, 2).reshape(128, -1))
    # x [p, b, sc, hk, 512]
    xT_a = _bf16(x.reshape(B, SC, 512, 8, 128).transpose(4, 0, 1, 3, 2)
                 .reshape(128, -1))
    in_maps = []
    for c in range(NC_):
        h0 = LH * c  # heads {2c, 2c+1}
        wq_loc = w_qkv[:, 64 * h0: 64 * h0 + LQK]
        wk_loc = w_qkv[:, H + 64 * h0: H + 64 * h0 + LQK]
        wv_loc = w_qkv[:, 2 * H + 64 * h0: 2 * H + 64 * h0 + LQK]
        wv_aug = np.zeros((H, 130), np.float32)
        for j in range(LH):
            wv_aug[:, 65 * j: 65 * j + 64] = wv_loc[:, 64 * j: 64 * j + 64]
        # [p, g(K|Q), hk, 128]
        wqk_g = np.stack([wk_loc.reshape(8, 128, 128),
                          wq_loc.reshape(8, 128, 128)], axis=0)
        wqk_a = _bf16(wqk_g.transpose(2, 0, 1, 3).reshape(128, -1))
        wv_a = _bf16(wv_aug.reshape(8, 128, 130).transpose(1, 0, 2)
                     .reshape(128, -1))
        in_maps.append({
            "xT": xT_a,
            "wqk": wqk_a,
            "wv": wv_a,
            "wout": wout_a,
            "cos2": cos2_a,
            "sinm": sinm_a,
        })
    return in_maps


def run(x, rotary_emb, w_qkv, w_out, trace=False, tmpdir=None):
    nc = _get_nc()
    in_maps = make_in_maps(x, rotary_emb, w_qkv, w_out)
    res = run_bass_kernel_spmd(nc, in_maps, list(range(NC_)), trace=trace,
                               tmpdir=tmpdir)
    full = np.empty((B, N, H), np.float32)
    for c in range(NC_):
        b, r = c // 4, c % 4
        piece = np.asarray(res.results[c]["out"]).astype(np.float32)
        for qc in range(QC):
            full[b, 512 * qc + 128 * r: 512 * qc + 128 * r + 128] = piece[qc]
    return full, res


def kernel(x, rotary_emb, w_qkv, w_out):
    full, _ = run(x, rotary_emb, w_qkv, w_out)
    return full
